# revision 1
# baseline (speedup 1.0000x reference)
"""DeepAR (2-layer LSTM, H=512) Trainium2 Bass kernel.

Full-input contract: kernel(**inputs) takes the unsharded inputs from
setup_inputs() and returns the full [512, 64, 2] output.  Internally the
batch (512) is sharded 64-per-core across 8 NeuronCores (data parallel);
LSTM weights are replicated.

Device strategy (per core, B=64):
  - All weights resident in SBUF, streamed through the PE as the MOVING
    matmul operand each timestep (float32r -> 1 col/cycle).  Stationary
    operands are transposed activations [K<=128, 64].
  - Gates accumulate in PSUM as [64(batch), 512] chunks (i, g, f, o).
  - LSTM1 bias b1 (and the +1 forget bias) are folded in via a ones-row
    appended to the xy feature chunk; LSTM2 bias b2 is added on VectorE.
  - h is transposed back to [H, B] tiles via PE transpose each step.
  - Autoregressive decode feeds m = h2 @ Wm + bm back into the feature
    row in-place in SBUF; mean/disp outputs accumulate in the same tile.
"""
import sys

sys.path.insert(0, "/opt/trn_rl_repo")

import numpy as np

import concourse.bass as bass
import concourse.mybir as mybir
from concourse import bass_utils, tile

F32 = mybir.dt.float32
F32R = mybir.dt.float32r
Act = mybir.ActivationFunctionType

B_FULL, TP, TO, F, H = 512, 192, 128, 64, 512
NC = 8
B = B_FULL // NC            # 64 per core
G = 4 * H                   # 2048 gate width
NSLOT = TP + 1              # 193 feature slots (slot t feeds step t)
XCOLS = NSLOT * B           # 12352


def ts(i, n):
    return slice(i * n, (i + 1) * n)


def split_excess_waits(nc):
    """Walrus accepts only one sync-wait per hardware instruction. Hoist
    excess waits onto NoOps (same engine) inserted right before."""
    n = 0
    for f in nc.m.functions:
        for blk in f.blocks:
            out = []
            for inst in blk.instructions:
                si = inst.sync_info
                if si is not None and si.on_wait and len(si.on_wait) > 1:
                    waits = list(si.on_wait)
                    for j, w in enumerate(waits[:-1]):
                        nop = mybir.InstNoOp(
                            name=f"{inst.name}-wnop{j}", ins=[], outs=[])
                        nop.engine = inst.engine
                        nop.sync_info = mybir.SyncInfo(on_wait=[w], on_update=[])
                        out.append(nop)
                        n += 1
                    inst.sync_info = mybir.SyncInfo(
                        on_wait=[waits[-1]], on_update=list(si.on_update))
                out.append(inst)
            blk.instructions = out
    return n


def build_program(tp=TP, to=TO, split_waits=True):
    NSLOT_ = tp + 1
    XCOLS_ = NSLOT_ * B
    nc = bass.Bass("TRN2", target_bir_lowering=False, debug=False,
                   num_devices=NC)

    xyf_d = nc.dram_tensor("xyf_d", [66, XCOLS_], F32R, kind="ExternalInput").ap()
    w1c0_d = nc.dram_tensor("w1c0_d", [66, G], F32R, kind="ExternalInput").ap()
    w1h_d = nc.dram_tensor("w1h_d", [128, 4 * G], F32R, kind="ExternalInput").ap()
    w2_d = nc.dram_tensor("w2_d", [128, 8 * G], F32R, kind="ExternalInput").ap()
    wmd_d = nc.dram_tensor("wmd_d", [128, 4 * 64], F32R, kind="ExternalInput").ap()
    b2r_d = nc.dram_tensor("b2r_d", [64, G], F32, kind="ExternalInput").ap()
    bmd_d = nc.dram_tensor("bmd_d", [33, 1], F32, kind="ExternalInput").ap()
    id_d = nc.dram_tensor("id_d", [64, 64], F32, kind="ExternalInput").ap()
    out_d = nc.dram_tensor("out_d", [2, (tp - to) * B], F32,
                           kind="ExternalOutput").ap()

    with tile.TileContext(nc) as tc:
        with tc.sbuf_pool(name="const", bufs=1) as cp, \
             tc.sbuf_pool(name="work", bufs=1) as wp, \
             tc.psum_pool(name="ps", bufs=1) as pp:
            # ---- persistent tiles + input DMA ----
            xyf = cp.tile([66, XCOLS_], F32R, name="xyf")
            w1c0 = cp.tile([66, G], F32R, name="w1c0")
            w1h = cp.tile([128, 4 * G], F32R, name="w1h")
            w2 = cp.tile([128, 8 * G], F32R, name="w2")
            wmd = cp.tile([128, 4 * 64], F32R, name="wmd")
            b2r = cp.tile([64, G], F32, name="b2r")
            bmd = cp.tile([33, 1], F32, name="bmd")
            ident = cp.tile([64, 64], F32, name="ident")

            nc.sync.dma_start(xyf[:, :], xyf_d[:, :])
            nc.sync.dma_start(w1c0[:, :], w1c0_d[:, :])
            for k in range(4):
                nc.sync.dma_start(w1h[:, ts(k, G)], w1h_d[:, ts(k, G)])
            for k in range(8):
                nc.sync.dma_start(w2[:, ts(k, G)], w2_d[:, ts(k, G)])
            nc.sync.dma_start(wmd[:, :], wmd_d[:, :])
            nc.sync.dma_start(b2r[:, :], b2r_d[:, :])
            nc.sync.dma_start(bmd[:, :], bmd_d[:, :])
            nc.sync.dma_start(ident[:, :], id_d[:, :])

            # ---- state tiles ----
            c1 = cp.tile([64, H], F32, name="c1")
            c2 = cp.tile([64, H], F32, name="c2")
            nc.vector.memset(c1[:, :], 0.0)
            nc.vector.memset(c2[:, :], 0.0)

            h1T_prev = None
            h2T_prev = None
            g1p_pend = None

            def lstm_post(gp, c_state, b2_tile, htag):
                """gates psum chunks -> h [64, H] sbuf tile (fp32)."""
                if b2_tile is None:
                    i_s = wp.tile([64, H], F32, name=f"i{htag}", tag=f"i{htag}")
                    g_s = wp.tile([64, H], F32, name=f"g{htag}", tag=f"g{htag}")
                    f_s = wp.tile([64, H], F32, name=f"f{htag}", tag=f"f{htag}")
                    o_s = wp.tile([64, H], F32, name=f"o{htag}", tag=f"o{htag}")
                    nc.scalar.activation(i_s[:, :], gp[0][:, :], Act.Sigmoid)
                    nc.scalar.activation(g_s[:, :], gp[1][:, :], Act.Tanh)
                    nc.scalar.activation(f_s[:, :], gp[2][:, :], Act.Sigmoid)
                    nc.scalar.activation(o_s[:, :], gp[3][:, :], Act.Sigmoid)
                else:
                    # bias-add on DVE first (b2 replicated across partitions)
                    acts = []
                    fns = [Act.Sigmoid, Act.Tanh, Act.Sigmoid, Act.Sigmoid]
                    names = ["i", "g", "f", "o"]
                    for j in range(4):
                        pre = wp.tile([64, H], F32, name=f"pre{htag}_{j}",
                                      tag=f"pre{htag}", bufs=2)
                        nc.vector.tensor_add(pre[:, :], gp[j][:, :],
                                             b2_tile[:, ts(j, H)])
                        s = wp.tile([64, H], F32, name=f"{names[j]}{htag}",
                                    tag=f"{names[j]}{htag}")
                        nc.scalar.activation(s[:, :], pre[:, :], fns[j])
                        acts.append(s)
                    i_s, g_s, f_s, o_s = acts
                t1 = wp.tile([64, H], F32, name=f"t1{htag}", tag=f"t1{htag}")
                t2 = wp.tile([64, H], F32, name=f"t2{htag}", tag=f"t2{htag}")
                nc.vector.tensor_mul(t1[:, :], i_s[:, :], g_s[:, :])
                nc.vector.tensor_mul(t2[:, :], f_s[:, :], c_state[:, :])
                nc.vector.tensor_add(c_state[:, :], t1[:, :], t2[:, :])
                tc_s = wp.tile([64, H], F32, name=f"tc{htag}", tag=f"t1{htag}",
                               bufs=1)
                nc.scalar.activation(tc_s[:, :], c_state[:, :], Act.Tanh)
                h = wp.tile([64, H], F32, name=f"h{htag}", tag=f"h{htag}")
                nc.vector.tensor_mul(h[:, :], o_s[:, :], tc_s[:, :])
                return h

            def transpose_h(h, htag):
                trp = pp.tile([128, 256], F32, name=f"tr{htag}", tag="small",
                              bufs=1)
                for kk in range(4):
                    nc.tensor.transpose(trp[:, ts(kk, 64)],
                                        h[:, ts(kk, 128)], ident[:, :])
                hT = wp.tile([128, 256], F32R, name=f"hT{htag}",
                             tag=f"hT{htag}", bufs=2)
                nc.vector.tensor_copy(hT[:, 0:128], trp[:, 0:128])
                nc.vector.tensor_copy(hT[:, 128:256], trp[:, 128:256])
                return hT

            for t in range(tp):
                first = t == 0
                # --- phase A: finish L1(t) gates with the xy chunk ---
                if g1p_pend is None:
                    g1p = [pp.tile([64, H], F32, name=f"g1p{j}", tag="g1",
                                   bufs=4) for j in range(4)]
                else:
                    g1p = g1p_pend
                # --- phase B first: L2(t) h2-part (no dependence on m) ---
                g2p = [pp.tile([64, H], F32, name=f"g2p{j}", tag="g2",
                               bufs=3) for j in range(4)]
                if not first:
                    for j in range(4):
                        for k in range(4):
                            nc.tensor.matmul(
                                g2p[j][:, :], h2T_prev[:, ts(k, 64)],
                                w2[:, (4 + k) * G + j * H:(4 + k) * G + (j + 1) * H],
                                start=(k == 0), stop=False,
                                skip_group_check=True)
                # --- phase A: finish L1(t) gates with the xy chunk (AR: waits m) ---
                for j in range(4):
                    nc.tensor.matmul(g1p[j][:, :], xyf[0:66, ts(t, 64)],
                                     w1c0[:, ts(j, H)], start=first,
                                     stop=True, skip_group_check=True)
                # --- phase C: L1 post + h1 transpose ---
                h1 = lstm_post(g1p, c1, None, "1")
                h1T = transpose_h(h1, "1")
                # --- phase D: L2(t) h1-part ---
                for j in range(4):
                    for k in range(4):
                        nc.tensor.matmul(
                            g2p[j][:, :], h1T[:, ts(k, 64)],
                            w2[:, k * G + j * H:k * G + (j + 1) * H],
                            start=(first and k == 0), stop=(k == 3),
                            skip_group_check=True)
                # --- phase E: L1(t+1) h-part (pipelined ahead) ---
                if t < tp - 1:
                    g1p_pend = [pp.tile([64, H], F32, name=f"g1pn{j}",
                                        tag="g1", bufs=4) for j in range(4)]
                    for j in range(4):
                        for k in range(4):
                            nc.tensor.matmul(
                                g1p_pend[j][:, :], h1T[:, ts(k, 64)],
                                w1h[:, k * G + j * H:k * G + (j + 1) * H],
                                start=(k == 0), stop=False,
                                skip_group_check=True)
                else:
                    g1p_pend = None
                # --- phase F: L2 post + h2 transpose ---
                h2 = lstm_post(g2p, c2, b2r, "2")
                h2T = transpose_h(h2, "2")
                # --- phase G: m/d head (AR feedback + outputs) ---
                if t >= to - 1:
                    mdp = pp.tile([64, 64], F32, name="mdp", tag="small",
                                  bufs=1)
                    for k in range(4):
                        nc.tensor.matmul(mdp[:, :], wmd[:, ts(k, 64)],
                                         h2T[:, ts(k, 64)], start=(k == 0),
                                         stop=(k == 3),
                                         skip_group_check=True)
                    # m -> feature row 0, slot t+1 (f32r rounding on write)
                    nc.scalar.activation(xyf[0:1, ts(t + 1, 64)],
                                         mdp[0:1, :], Act.Identity,
                                         bias=bmd[0:1, 0:1], scale=1.0)
                    if t >= to:
                        # d -> row 64 (ones/d row), slot t (already consumed)
                        nc.scalar.activation(xyf[64:65, ts(t, 64)],
                                             mdp[32:33, :], Act.Identity,
                                             bias=bmd[32:33, 0:1], scale=1.0)
                h1T_prev, h2T_prev = h1T, h2T

            # ---- outputs: mean row = slots TO+1..TP, disp row = slots TO..TP-1
            nc.sync.dma_start(out_d[0:1, :],
                              xyf[0:1, (to + 1) * B:(tp + 1) * B].bitcast(F32))
            nc.sync.dma_start(out_d[1:2, :],
                              xyf[64:65, to * B:tp * B].bitcast(F32))

    n = split_excess_waits(nc) if split_waits else 0
    return nc, n


_CACHE = {}


def _get_program():
    if "nc" not in _CACHE:
        _CACHE["nc"] = build_program()[0]
    return _CACHE["nc"]


def make_core_inputs(x, y, W1, b1, W2, b2, Wm, bm, Wd, bd, tp=TP, to=TO):
    """Host-side prep: returns (in_maps list of 8 dicts, scale [512])."""
    NSLOT_ = tp + 1
    XCOLS_ = NSLOT_ * B
    x = np.asarray(x, np.float32)
    y = np.asarray(y, np.float32)
    W1 = np.asarray(W1, np.float32)
    b1 = np.asarray(b1, np.float32)
    W2 = np.asarray(W2, np.float32)
    b2 = np.asarray(b2, np.float32)
    Wm = np.asarray(Wm, np.float32)
    bm = np.asarray(bm, np.float32)
    Wd = np.asarray(Wd, np.float32)
    bd = np.asarray(bd, np.float32)

    scale = 1.0 + np.mean(y[:, 0:to, 0], axis=1)       # [512]
    y_sc = y[:, 0:to, 0] / scale[:, None]              # [512, to]

    b1a = b1.copy()
    b1a[2 * H:3 * H] += 1.0                             # forget-gate +1
    b2a = b2.copy()
    b2a[2 * H:3 * H] += 1.0

    # row layout: 0 = y/m, 1:64 = x[0:63], 64 = ones/bias (disp storage),
    # 65 = x[63]  (rows 0 and 64 must sit at legal engine partition bases)
    w1c0 = np.empty((66, G), np.float32)
    w1c0[0] = W1[F]                                     # y/m weight row
    w1c0[1:64] = W1[0:F - 1]                            # x weight rows 0..62
    w1c0[64] = b1a                                      # bias row (ones input)
    w1c0[65] = W1[F - 1]                                # x weight row 63

    w1h = np.ascontiguousarray(
        W1[F + 1:].reshape(4, 128, G).transpose(1, 0, 2).reshape(128, 4 * G))
    w2 = np.ascontiguousarray(
        W2.reshape(8, 128, G).transpose(1, 0, 2).reshape(128, 8 * G))

    wmd = np.zeros((128, 4, 64), np.float32)
    wmd[:, :, 0] = Wm[:, 0].reshape(4, 128).T
    wmd[:, :, 32] = Wd[:, 0].reshape(4, 128).T
    wmd = np.ascontiguousarray(wmd.reshape(128, 4 * 64))

    b2rep = np.ascontiguousarray(np.broadcast_to(b2a, (64, G)))
    bmd = np.zeros((33, 1), np.float32)
    bmd[0, 0] = bm[0]
    bmd[32, 0] = bd[0]
    identity = np.eye(64, dtype=np.float32)

    in_maps = []
    for c in range(NC):
        bs = slice(c * B, (c + 1) * B)
        xyf = np.zeros((66, NSLOT_, B), np.float32)
        xyf[0, 1:to, :] = y_sc[bs, 0:to - 1].T          # shifted y feed
        xt = x[bs].transpose(2, 1, 0)                   # [f, t, b]
        xyf[1:64, 0:tp, :] = xt[0:F - 1]                # x rows 0..62
        xyf[65, 0:tp, :] = xt[F - 1]                    # x row 63
        xyf[64, :, :] = 1.0                             # ones / bias row
        in_maps.append({
            "xyf_d": np.ascontiguousarray(xyf.reshape(66, XCOLS_)),
            "w1c0_d": w1c0, "w1h_d": w1h, "w2_d": w2, "wmd_d": wmd,
            "b2r_d": b2rep, "bmd_d": bmd, "id_d": identity,
        })
    return in_maps, scale


def postprocess(results, scale, tp=TP, to=TO):
    """results: list of 8 dicts with out_d [2, (tp-to)*64] -> [512, tp-to, 2]."""
    out = np.empty((B_FULL, tp - to, 2), np.float32)
    for c in range(NC):
        r = results[c]["out_d"]
        mean_tb = r[0].reshape(tp - to, B)              # [t, b]
        dpre_tb = r[1].reshape(tp - to, B)
        bs = slice(c * B, (c + 1) * B)
        sc = scale[bs]
        out[bs, :, 0] = (mean_tb * sc[None, :]).T
        disp = np.logaddexp(dpre_tb, 0.0)               # softplus
        out[bs, :, 1] = (disp * np.sqrt(sc)[None, :]).T
    return out


def kernel(x, y, W1, b1, W2, b2, Wm, bm, Wd, bd):
    in_maps, scale = make_core_inputs(x, y, W1, b1, W2, b2, Wm, bm, Wd, bd)
    nc = _get_program()
    res = bass_utils.run_bass_kernel_spmd(nc, in_maps, core_ids=list(range(NC)))
    return postprocess(res.results, scale)



# revision 3
# speedup vs baseline: 1.4465x; 1.4465x over previous
"""DeepAR (2-layer LSTM, H=512) Trainium2 Bass kernel.

Full-input contract: kernel(**inputs) takes the unsharded inputs from
setup_inputs() and returns the full [512, 64, 2] output.  Internally the
batch (512) is sharded 64-per-core across 8 NeuronCores (data parallel);
LSTM weights are replicated.

Device strategy (per core, B=64):
  - All weights resident in SBUF, streamed through the PE as the MOVING
    matmul operand each timestep (fp16 -> 1 col/cycle + fast weight
    load; fp32r on real TRN2 runs ~4x slower).  Stationary operands are
    transposed activations [K<=128, 64] in fp16.
  - Gates accumulate in PSUM (fp32) as [64(batch), 512] chunks
    (i, g, f, o).  Cell state c and gate activations stay fp32.
  - LSTM1 bias b1 (and the +1 forget bias) are folded in via a ones-row
    appended to the xy feature chunk; LSTM2 bias b2 is added on VectorE.
  - h is written fp16, transposed back to [H, B] tiles via PE transpose
    each step.
  - Autoregressive decode feeds m = h2 @ Wm + bm back into the feature
    row in-place in SBUF; mean/disp outputs accumulate in the same tile.
"""
import sys

sys.path.insert(0, "/opt/trn_rl_repo")

import numpy as np

import concourse.bass as bass
import concourse.mybir as mybir
from concourse import bass_utils, tile

F32 = mybir.dt.float32
F32R = mybir.dt.float32r
F16 = mybir.dt.float16
Act = mybir.ActivationFunctionType

B_FULL, TP, TO, F, H = 512, 192, 128, 64, 512
NC = 8
B = B_FULL // NC            # 64 per core
G = 4 * H                   # 2048 gate width
NSLOT = TP + 1              # 193 feature slots (slot t feeds step t)
XCOLS = NSLOT * B           # 12352


def ts(i, n):
    return slice(i * n, (i + 1) * n)


def split_excess_waits(nc):
    """Walrus accepts only one sync-wait per hardware instruction. Hoist
    excess waits onto NoOps (same engine) inserted right before."""
    n = 0
    for f in nc.m.functions:
        for blk in f.blocks:
            out = []
            for inst in blk.instructions:
                si = inst.sync_info
                if si is not None and si.on_wait and len(si.on_wait) > 1:
                    waits = list(si.on_wait)
                    for j, w in enumerate(waits[:-1]):
                        nop = mybir.InstNoOp(
                            name=f"{inst.name}-wnop{j}", ins=[], outs=[])
                        nop.engine = inst.engine
                        nop.sync_info = mybir.SyncInfo(on_wait=[w], on_update=[])
                        out.append(nop)
                        n += 1
                    inst.sync_info = mybir.SyncInfo(
                        on_wait=[waits[-1]], on_update=list(si.on_update))
                out.append(inst)
            blk.instructions = out
    return n


def build_program(tp=TP, to=TO, split_waits=True, mm_dt=F16):
    NSLOT_ = tp + 1
    XCOLS_ = NSLOT_ * B
    nc = bass.Bass("TRN2", target_bir_lowering=False, debug=False,
                   num_devices=NC)

    xyf_d = nc.dram_tensor("xyf_d", [66, XCOLS_], mm_dt, kind="ExternalInput").ap()
    w1c0_d = nc.dram_tensor("w1c0_d", [66, G], mm_dt, kind="ExternalInput").ap()
    w1h_d = nc.dram_tensor("w1h_d", [128, 4 * G], mm_dt, kind="ExternalInput").ap()
    w2_d = nc.dram_tensor("w2_d", [128, 8 * G], mm_dt, kind="ExternalInput").ap()
    wmd_d = nc.dram_tensor("wmd_d", [128, 4 * 64], mm_dt, kind="ExternalInput").ap()
    b2r_d = nc.dram_tensor("b2r_d", [64, G], F32, kind="ExternalInput").ap()
    bmd_d = nc.dram_tensor("bmd_d", [33, 1], F32, kind="ExternalInput").ap()
    id_d = nc.dram_tensor("id_d", [64, 64], mm_dt, kind="ExternalInput").ap()
    out_d = nc.dram_tensor("out_d", [2, (tp - to) * B], mm_dt,
                           kind="ExternalOutput").ap()

    with tile.TileContext(nc) as tc:
        with tc.sbuf_pool(name="const", bufs=1) as cp, \
             tc.sbuf_pool(name="work", bufs=1) as wp, \
             tc.psum_pool(name="ps", bufs=1) as pp:
            # ---- persistent tiles + input DMA ----
            xyf = cp.tile([66, XCOLS_], mm_dt, name="xyf")
            w1c0 = cp.tile([66, G], mm_dt, name="w1c0")
            w1h = cp.tile([128, 4 * G], mm_dt, name="w1h")
            w2 = cp.tile([128, 8 * G], mm_dt, name="w2")
            wmd = cp.tile([128, 4 * 64], mm_dt, name="wmd")
            b2r = cp.tile([64, G], F32, name="b2r")
            bmd = cp.tile([33, 1], F32, name="bmd")
            ident = cp.tile([64, 64], mm_dt, name="ident")

            nc.sync.dma_start(xyf[:, :], xyf_d[:, :])
            nc.sync.dma_start(w1c0[:, :], w1c0_d[:, :])
            for k in range(4):
                nc.sync.dma_start(w1h[:, ts(k, G)], w1h_d[:, ts(k, G)])
            for k in range(8):
                nc.sync.dma_start(w2[:, ts(k, G)], w2_d[:, ts(k, G)])
            nc.sync.dma_start(wmd[:, :], wmd_d[:, :])
            nc.sync.dma_start(b2r[:, :], b2r_d[:, :])
            nc.sync.dma_start(bmd[:, :], bmd_d[:, :])
            nc.sync.dma_start(ident[:, :], id_d[:, :])

            # ---- state tiles ----
            c1 = cp.tile([64, H], F32, name="c1")
            c2 = cp.tile([64, H], F32, name="c2")
            nc.vector.memset(c1[:, :], 0.0)
            nc.vector.memset(c2[:, :], 0.0)

            h1T_prev = None
            h2T_prev = None
            g1p_pend = None

            def lstm_post(gp, c_state, b2_tile, htag):
                """gates psum chunks -> h [64, H] sbuf tile (fp16)."""
                if b2_tile is None:
                    i_s = wp.tile([64, H], F32, name=f"i{htag}", tag=f"i{htag}")
                    g_s = wp.tile([64, H], F32, name=f"g{htag}", tag=f"g{htag}")
                    f_s = wp.tile([64, H], F32, name=f"f{htag}", tag=f"f{htag}")
                    o_s = wp.tile([64, H], F32, name=f"o{htag}", tag=f"o{htag}")
                    nc.scalar.activation(i_s[:, :], gp[0][:, :], Act.Sigmoid)
                    nc.scalar.activation(g_s[:, :], gp[1][:, :], Act.Tanh)
                    nc.scalar.activation(f_s[:, :], gp[2][:, :], Act.Sigmoid)
                    nc.scalar.activation(o_s[:, :], gp[3][:, :], Act.Sigmoid)
                else:
                    # bias-add on DVE first (b2 replicated across partitions)
                    acts = []
                    fns = [Act.Sigmoid, Act.Tanh, Act.Sigmoid, Act.Sigmoid]
                    names = ["i", "g", "f", "o"]
                    for j in range(4):
                        pre = wp.tile([64, H], F32, name=f"pre{htag}_{j}",
                                      tag=f"pre{htag}", bufs=2)
                        nc.vector.tensor_add(pre[:, :], gp[j][:, :],
                                             b2_tile[:, ts(j, H)])
                        s = wp.tile([64, H], F32, name=f"{names[j]}{htag}",
                                    tag=f"{names[j]}{htag}")
                        nc.scalar.activation(s[:, :], pre[:, :], fns[j])
                        acts.append(s)
                    i_s, g_s, f_s, o_s = acts
                t1 = wp.tile([64, H], F32, name=f"t1{htag}", tag=f"t1{htag}")
                t2 = wp.tile([64, H], F32, name=f"t2{htag}", tag=f"t2{htag}")
                nc.vector.tensor_mul(t1[:, :], i_s[:, :], g_s[:, :])
                nc.vector.tensor_mul(t2[:, :], f_s[:, :], c_state[:, :])
                nc.vector.tensor_add(c_state[:, :], t1[:, :], t2[:, :])
                tc_s = wp.tile([64, H], F32, name=f"tc{htag}", tag=f"t1{htag}",
                               bufs=1)
                nc.scalar.activation(tc_s[:, :], c_state[:, :], Act.Tanh)
                h = wp.tile([64, H], mm_dt, name=f"h{htag}", tag=f"h{htag}")
                nc.vector.tensor_mul(h[:, :], o_s[:, :], tc_s[:, :])
                return h

            def transpose_h(h, htag):
                trp = pp.tile([128, 256], mm_dt, name=f"tr{htag}", tag="small",
                              bufs=1)
                for kk in range(4):
                    nc.tensor.transpose(trp[:, ts(kk, 64)],
                                        h[:, ts(kk, 128)], ident[:, :])
                hT = wp.tile([128, 256], mm_dt, name=f"hT{htag}",
                             tag=f"hT{htag}", bufs=2)
                nc.vector.tensor_copy(hT[:, 0:128], trp[:, 0:128])
                nc.vector.tensor_copy(hT[:, 128:256], trp[:, 128:256])
                return hT

            for t in range(tp):
                first = t == 0
                # --- phase A: finish L1(t) gates with the xy chunk ---
                if g1p_pend is None:
                    g1p = [pp.tile([64, H], F32, name=f"g1p{j}", tag="g1",
                                   bufs=4) for j in range(4)]
                else:
                    g1p = g1p_pend
                # --- phase B first: L2(t) h2-part (no dependence on m) ---
                g2p = [pp.tile([64, H], F32, name=f"g2p{j}", tag="g2",
                               bufs=3) for j in range(4)]
                if not first:
                    for j in range(4):
                        for k in range(4):
                            nc.tensor.matmul(
                                g2p[j][:, :], h2T_prev[:, ts(k, 64)],
                                w2[:, (4 + k) * G + j * H:(4 + k) * G + (j + 1) * H],
                                start=(k == 0), stop=False,
                                skip_group_check=True)
                # --- phase A: finish L1(t) gates with the xy chunk (AR: waits m) ---
                for j in range(4):
                    nc.tensor.matmul(g1p[j][:, :], xyf[0:66, ts(t, 64)],
                                     w1c0[:, ts(j, H)], start=first,
                                     stop=True, skip_group_check=True)
                # --- phase C: L1 post + h1 transpose ---
                h1 = lstm_post(g1p, c1, None, "1")
                h1T = transpose_h(h1, "1")
                # --- phase D: L2(t) h1-part ---
                for j in range(4):
                    for k in range(4):
                        nc.tensor.matmul(
                            g2p[j][:, :], h1T[:, ts(k, 64)],
                            w2[:, k * G + j * H:k * G + (j + 1) * H],
                            start=(first and k == 0), stop=(k == 3),
                            skip_group_check=True)
                # --- phase E: L1(t+1) h-part (pipelined ahead) ---
                if t < tp - 1:
                    g1p_pend = [pp.tile([64, H], F32, name=f"g1pn{j}",
                                        tag="g1", bufs=4) for j in range(4)]
                    for j in range(4):
                        for k in range(4):
                            nc.tensor.matmul(
                                g1p_pend[j][:, :], h1T[:, ts(k, 64)],
                                w1h[:, k * G + j * H:k * G + (j + 1) * H],
                                start=(k == 0), stop=False,
                                skip_group_check=True)
                else:
                    g1p_pend = None
                # --- phase F: L2 post + h2 transpose ---
                h2 = lstm_post(g2p, c2, b2r, "2")
                h2T = transpose_h(h2, "2")
                # --- phase G: m/d head (AR feedback + outputs) ---
                if t >= to - 1:
                    mdp = pp.tile([64, 64], F32, name="mdp", tag="small",
                                  bufs=1)
                    for k in range(4):
                        nc.tensor.matmul(mdp[:, :], wmd[:, ts(k, 64)],
                                         h2T[:, ts(k, 64)], start=(k == 0),
                                         stop=(k == 3),
                                         skip_group_check=True)
                    # m -> feature row 0, slot t+1 (fp16 rounding on write)
                    nc.scalar.activation(xyf[0:1, ts(t + 1, 64)],
                                         mdp[0:1, :], Act.Identity,
                                         bias=bmd[0:1, 0:1], scale=1.0)
                    if t >= to:
                        # d -> row 64 (ones/d row), slot t (already consumed)
                        nc.scalar.activation(xyf[64:65, ts(t, 64)],
                                             mdp[32:33, :], Act.Identity,
                                             bias=bmd[32:33, 0:1], scale=1.0)
                h1T_prev, h2T_prev = h1T, h2T

            # ---- outputs: mean row = slots TO+1..TP, disp row = slots TO..TP-1
            nc.sync.dma_start(out_d[0:1, :],
                              xyf[0:1, (to + 1) * B:(tp + 1) * B])
            nc.sync.dma_start(out_d[1:2, :],
                              xyf[64:65, to * B:tp * B])

    n = split_excess_waits(nc) if split_waits else 0
    return nc, n


_CACHE = {}


def _get_program():
    if "nc" not in _CACHE:
        _CACHE["nc"] = build_program()[0]
    return _CACHE["nc"]


def make_core_inputs(x, y, W1, b1, W2, b2, Wm, bm, Wd, bd, tp=TP, to=TO,
                     np_dt=np.float16):
    """Host-side prep: returns (in_maps list of 8 dicts, scale [512])."""
    NSLOT_ = tp + 1
    XCOLS_ = NSLOT_ * B
    x = np.asarray(x, np.float32)
    y = np.asarray(y, np.float32)
    W1 = np.asarray(W1, np.float32)
    b1 = np.asarray(b1, np.float32)
    W2 = np.asarray(W2, np.float32)
    b2 = np.asarray(b2, np.float32)
    Wm = np.asarray(Wm, np.float32)
    bm = np.asarray(bm, np.float32)
    Wd = np.asarray(Wd, np.float32)
    bd = np.asarray(bd, np.float32)

    scale = 1.0 + np.mean(y[:, 0:to, 0], axis=1)       # [512]
    y_sc = y[:, 0:to, 0] / scale[:, None]              # [512, to]

    b1a = b1.copy()
    b1a[2 * H:3 * H] += 1.0                             # forget-gate +1
    b2a = b2.copy()
    b2a[2 * H:3 * H] += 1.0

    # row layout: 0 = y/m, 1:64 = x[0:63], 64 = ones/bias (disp storage),
    # 65 = x[63]  (rows 0 and 64 must sit at legal engine partition bases)
    w1c0 = np.empty((66, G), np.float32)
    w1c0[0] = W1[F]                                     # y/m weight row
    w1c0[1:64] = W1[0:F - 1]                            # x weight rows 0..62
    w1c0[64] = b1a                                      # bias row (ones input)
    w1c0[65] = W1[F - 1]                                # x weight row 63

    w1h = np.ascontiguousarray(
        W1[F + 1:].reshape(4, 128, G).transpose(1, 0, 2).reshape(128, 4 * G))
    w2 = np.ascontiguousarray(
        W2.reshape(8, 128, G).transpose(1, 0, 2).reshape(128, 8 * G))

    wmd = np.zeros((128, 4, 64), np.float32)
    wmd[:, :, 0] = Wm[:, 0].reshape(4, 128).T
    wmd[:, :, 32] = Wd[:, 0].reshape(4, 128).T
    wmd = np.ascontiguousarray(wmd.reshape(128, 4 * 64))

    b2rep = np.ascontiguousarray(np.broadcast_to(b2a, (64, G)))
    bmd = np.zeros((33, 1), np.float32)
    bmd[0, 0] = bm[0]
    bmd[32, 0] = bd[0]
    identity = np.eye(64, dtype=np_dt)

    in_maps = []
    for c in range(NC):
        bs = slice(c * B, (c + 1) * B)
        xyf = np.zeros((66, NSLOT_, B), np.float32)
        xyf[0, 1:to, :] = y_sc[bs, 0:to - 1].T          # shifted y feed
        xt = x[bs].transpose(2, 1, 0)                   # [f, t, b]
        xyf[1:64, 0:tp, :] = xt[0:F - 1]                # x rows 0..62
        xyf[65, 0:tp, :] = xt[F - 1]                    # x row 63
        xyf[64, :, :] = 1.0                             # ones / bias row
        in_maps.append({
            "xyf_d": np.ascontiguousarray(xyf.reshape(66, XCOLS_)).astype(np_dt),
            "w1c0_d": w1c0.astype(np_dt), "w1h_d": w1h.astype(np_dt),
            "w2_d": w2.astype(np_dt), "wmd_d": wmd.astype(np_dt),
            "b2r_d": b2rep, "bmd_d": bmd, "id_d": identity,
        })
    return in_maps, scale


def postprocess(results, scale, tp=TP, to=TO):
    """results: list of 8 dicts with out_d [2, (tp-to)*64] -> [512, tp-to, 2]."""
    out = np.empty((B_FULL, tp - to, 2), np.float32)
    for c in range(NC):
        r = results[c]["out_d"].astype(np.float32)
        mean_tb = r[0].reshape(tp - to, B)              # [t, b]
        dpre_tb = r[1].reshape(tp - to, B)
        bs = slice(c * B, (c + 1) * B)
        sc = scale[bs]
        out[bs, :, 0] = (mean_tb * sc[None, :]).T
        disp = np.logaddexp(dpre_tb, 0.0)               # softplus
        out[bs, :, 1] = (disp * np.sqrt(sc)[None, :]).T
    return out


def kernel(x, y, W1, b1, W2, b2, Wm, bm, Wd, bd):
    in_maps, scale = make_core_inputs(x, y, W1, b1, W2, b2, Wm, bm, Wd, bd)
    nc = _get_program()
    res = bass_utils.run_bass_kernel_spmd(nc, in_maps, core_ids=list(range(NC)))
    return postprocess(res.results, scale)


# revision 6
# speedup vs baseline: 20.9823x; 14.5060x over previous
"""DeepAR (2-layer LSTM, H=512) Trainium2 Bass kernel.

Full-input contract: kernel(**inputs) takes the unsharded inputs from
setup_inputs() and returns the full [512, 64, 2] output.  Internally the
batch (512) is sharded 64-per-core across 8 NeuronCores (data parallel);
LSTM weights are replicated.

Device strategy (per core, B=64):
  - All weights resident in SBUF, streamed through the PE as the MOVING
    matmul operand each timestep (fp16 -> 1 col/cycle + fast weight
    load; fp32r on real TRN2 runs ~4x slower).  Stationary operands are
    transposed activations [K<=128, 64] in fp16.
  - Gates accumulate in PSUM (fp32) as [64(batch), 512] chunks
    (i, g, f, o).  Cell state c and gate activations stay fp32.
  - LSTM1 bias b1 (and the +1 forget bias) are folded in via a ones-row
    appended to the xy feature chunk; LSTM2 bias b2 is added on VectorE.
  - h is written fp16, transposed back to [H, B] tiles via PE transpose
    each step.
  - Autoregressive decode feeds m = h2 @ Wm + bm back into the feature
    row in-place in SBUF; mean/disp outputs accumulate in the same tile.
"""
import sys

sys.path.insert(0, "/opt/trn_rl_repo")

import numpy as np

import concourse.bass as bass
import concourse.mybir as mybir
from concourse import bass_utils, tile

F32 = mybir.dt.float32
F32R = mybir.dt.float32r
F16 = mybir.dt.float16
Act = mybir.ActivationFunctionType

B_FULL, TP, TO, F, H = 512, 192, 128, 64, 512
NC = 8
B = B_FULL // NC            # 64 per core
G = 4 * H                   # 2048 gate width
NSLOT = TP + 1              # 193 feature slots (slot t feeds step t)
XCOLS = NSLOT * B           # 12352


def ts(i, n):
    return slice(i * n, (i + 1) * n)


def split_excess_waits(nc):
    """Walrus accepts only one sync-wait per hardware instruction. Hoist
    excess waits onto NoOps (same engine) inserted right before."""
    n = 0
    for f in nc.m.functions:
        for blk in f.blocks:
            out = []
            for inst in blk.instructions:
                si = inst.sync_info
                if si is not None and si.on_wait and len(si.on_wait) > 1:
                    waits = list(si.on_wait)
                    for j, w in enumerate(waits[:-1]):
                        nop = mybir.InstNoOp(
                            name=f"{inst.name}-wnop{j}", ins=[], outs=[])
                        nop.engine = inst.engine
                        nop.sync_info = mybir.SyncInfo(on_wait=[w], on_update=[])
                        out.append(nop)
                        n += 1
                    inst.sync_info = mybir.SyncInfo(
                        on_wait=[waits[-1]], on_update=list(si.on_update))
                out.append(inst)
            blk.instructions = out
    return n


def build_program(tp=TP, to=TO, split_waits=True, mm_dt=F16, post_mode="full"):
    NSLOT_ = tp + 1
    XCOLS_ = NSLOT_ * B
    nc = bass.Bass("TRN2", target_bir_lowering=False, debug=False,
                   num_devices=NC)

    xyf_d = nc.dram_tensor("xyf_d", [66, XCOLS_], mm_dt, kind="ExternalInput").ap()
    w1c0_d = nc.dram_tensor("w1c0_d", [66, G], mm_dt, kind="ExternalInput").ap()
    w1h_d = nc.dram_tensor("w1h_d", [128, 4 * G], mm_dt, kind="ExternalInput").ap()
    w2_d = nc.dram_tensor("w2_d", [128, 8 * G], mm_dt, kind="ExternalInput").ap()
    wmd_d = nc.dram_tensor("wmd_d", [128, 4 * 64], mm_dt, kind="ExternalInput").ap()
    b2r_d = nc.dram_tensor("b2r_d", [64, G], F32, kind="ExternalInput").ap()
    bmd_d = nc.dram_tensor("bmd_d", [33, 1], F32, kind="ExternalInput").ap()
    id_d = nc.dram_tensor("id_d", [64, 64], mm_dt, kind="ExternalInput").ap()
    out_d = nc.dram_tensor("out_d", [2, (tp - to) * B], mm_dt,
                           kind="ExternalOutput").ap()

    with tile.TileContext(nc) as tc:
        with tc.sbuf_pool(name="const", bufs=1) as cp, \
             tc.sbuf_pool(name="work", bufs=1) as wp, \
             tc.psum_pool(name="ps", bufs=1) as pp:
            # ---- persistent tiles + input DMA ----
            xyf = cp.tile([66, XCOLS_], mm_dt, name="xyf")
            w1c0 = cp.tile([66, G], mm_dt, name="w1c0")
            w1h = cp.tile([128, 4 * G], mm_dt, name="w1h")
            w2 = cp.tile([128, 8 * G], mm_dt, name="w2")
            wmd = cp.tile([128, 4 * 64], mm_dt, name="wmd")
            b2r = cp.tile([64, G], F32, name="b2r")
            bmd = cp.tile([33, 1], F32, name="bmd")
            ident = cp.tile([64, 64], mm_dt, name="ident")

            nc.sync.dma_start(xyf[:, :], xyf_d[:, :])
            nc.sync.dma_start(w1c0[:, :], w1c0_d[:, :])
            for k in range(4):
                nc.sync.dma_start(w1h[:, ts(k, G)], w1h_d[:, ts(k, G)])
            for k in range(8):
                nc.sync.dma_start(w2[:, ts(k, G)], w2_d[:, ts(k, G)])
            nc.sync.dma_start(wmd[:, :], wmd_d[:, :])
            nc.sync.dma_start(b2r[:, :], b2r_d[:, :])
            nc.sync.dma_start(bmd[:, :], bmd_d[:, :])
            nc.sync.dma_start(ident[:, :], id_d[:, :])

            # ---- state tiles ----
            c1 = cp.tile([64, H], F32, name="c1")
            c2 = cp.tile([64, H], F32, name="c2")
            nc.vector.memset(c1[:, :], 0.0)
            nc.vector.memset(c2[:, :], 0.0)

            h1T_prev = None
            h2T_prev = None
            g1p_pend = None

            def lstm_post(gp, c_state, b2_tile, htag):
                """gates psum chunks -> h [64, H] sbuf tile (fp16)."""
                if post_mode == "min":
                    h = wp.tile([64, H], mm_dt, name=f"h{htag}", tag=f"h{htag}")
                    nc.scalar.activation(h[:, :], gp[0][:, :], Act.Sigmoid)
                    return h
                if b2_tile is None:
                    i_s = wp.tile([64, H], F32, name=f"i{htag}", tag=f"i{htag}")
                    g_s = wp.tile([64, H], F32, name=f"g{htag}", tag=f"g{htag}")
                    f_s = wp.tile([64, H], F32, name=f"f{htag}", tag=f"f{htag}")
                    o_s = wp.tile([64, H], F32, name=f"o{htag}", tag=f"o{htag}")
                    nc.scalar.activation(i_s[:, :], gp[0][:, :], Act.Sigmoid)
                    nc.scalar.activation(g_s[:, :], gp[1][:, :], Act.Tanh)
                    nc.scalar.activation(f_s[:, :], gp[2][:, :], Act.Sigmoid)
                    nc.scalar.activation(o_s[:, :], gp[3][:, :], Act.Sigmoid)
                else:
                    # bias-add on DVE first (b2 replicated across partitions)
                    acts = []
                    fns = [Act.Sigmoid, Act.Tanh, Act.Sigmoid, Act.Sigmoid]
                    names = ["i", "g", "f", "o"]
                    for j in range(4):
                        pre = wp.tile([64, H], F32, name=f"pre{htag}_{j}",
                                      tag=f"pre{htag}", bufs=2)
                        nc.vector.tensor_add(pre[:, :], gp[j][:, :],
                                             b2_tile[:, ts(j, H)])
                        s = wp.tile([64, H], F32, name=f"{names[j]}{htag}",
                                    tag=f"{names[j]}{htag}")
                        nc.scalar.activation(s[:, :], pre[:, :], fns[j])
                        acts.append(s)
                    i_s, g_s, f_s, o_s = acts
                t1 = wp.tile([64, H], F32, name=f"t1{htag}", tag=f"t1{htag}")
                t2 = wp.tile([64, H], F32, name=f"t2{htag}", tag=f"t2{htag}")
                nc.vector.tensor_mul(t1[:, :], i_s[:, :], g_s[:, :])
                nc.vector.tensor_mul(t2[:, :], f_s[:, :], c_state[:, :])
                nc.vector.tensor_add(c_state[:, :], t1[:, :], t2[:, :])
                tc_s = wp.tile([64, H], F32, name=f"tc{htag}", tag=f"t1{htag}",
                               bufs=1)
                nc.scalar.activation(tc_s[:, :], c_state[:, :], Act.Tanh)
                h = wp.tile([64, H], mm_dt, name=f"h{htag}", tag=f"h{htag}")
                nc.vector.tensor_mul(h[:, :], o_s[:, :], tc_s[:, :])
                return h

            def transpose_h(h, htag):
                trp = pp.tile([128, 256], mm_dt, name=f"tr{htag}", tag="small",
                              bufs=1)
                for kk in range(4):
                    nc.tensor.transpose(trp[:, ts(kk, 64)],
                                        h[:, ts(kk, 128)], ident[:, :])
                hT = wp.tile([128, 256], mm_dt, name=f"hT{htag}",
                             tag=f"hT{htag}", bufs=2)
                nc.vector.tensor_copy(hT[:, 0:128], trp[:, 0:128])
                nc.vector.tensor_copy(hT[:, 128:256], trp[:, 128:256])
                return hT

            for t in range(tp):
                first = t == 0
                # --- phase A: finish L1(t) gates with the xy chunk ---
                if g1p_pend is None:
                    g1p = [pp.tile([64, H], F32, name=f"g1p{j}", tag="g1",
                                   bufs=4) for j in range(4)]
                else:
                    g1p = g1p_pend
                # --- phase B first: L2(t) h2-part (no dependence on m) ---
                g2p = [pp.tile([64, H], F32, name=f"g2p{j}", tag="g2",
                               bufs=3) for j in range(4)]
                if not first:
                    for j in range(4):
                        for k in range(4):
                            nc.tensor.matmul(
                                g2p[j][:, :], h2T_prev[:, ts(k, 64)],
                                w2[:, (4 + k) * G + j * H:(4 + k) * G + (j + 1) * H],
                                start=(k == 0), stop=False,
                                skip_group_check=True)
                # --- phase A: finish L1(t) gates with the xy chunk (AR: waits m) ---
                for j in range(4):
                    nc.tensor.matmul(g1p[j][:, :], xyf[0:66, ts(t, 64)],
                                     w1c0[:, ts(j, H)], start=first,
                                     stop=True, skip_group_check=True)
                # --- phase C: L1 post + h1 transpose ---
                h1 = lstm_post(g1p, c1, None, "1")
                h1T = transpose_h(h1, "1")
                # --- phase D: L2(t) h1-part ---
                for j in range(4):
                    for k in range(4):
                        nc.tensor.matmul(
                            g2p[j][:, :], h1T[:, ts(k, 64)],
                            w2[:, k * G + j * H:k * G + (j + 1) * H],
                            start=(first and k == 0), stop=(k == 3),
                            skip_group_check=True)
                # --- phase E: L1(t+1) h-part (pipelined ahead) ---
                if t < tp - 1:
                    g1p_pend = [pp.tile([64, H], F32, name=f"g1pn{j}",
                                        tag="g1", bufs=4) for j in range(4)]
                    for j in range(4):
                        for k in range(4):
                            nc.tensor.matmul(
                                g1p_pend[j][:, :], h1T[:, ts(k, 64)],
                                w1h[:, k * G + j * H:k * G + (j + 1) * H],
                                start=(k == 0), stop=False,
                                skip_group_check=True)
                else:
                    g1p_pend = None
                # --- phase F: L2 post + h2 transpose ---
                h2 = lstm_post(g2p, c2, b2r, "2")
                h2T = transpose_h(h2, "2")
                # --- phase G: m/d head (AR feedback + outputs) ---
                if t >= to - 1:
                    mdp = pp.tile([64, 64], F32, name="mdp", tag="small",
                                  bufs=1)
                    for k in range(4):
                        nc.tensor.matmul(mdp[:, :], wmd[:, ts(k, 64)],
                                         h2T[:, ts(k, 64)], start=(k == 0),
                                         stop=(k == 3),
                                         skip_group_check=True)
                    # m -> feature row 0, slot t+1 (fp16 rounding on write)
                    nc.scalar.activation(xyf[0:1, ts(t + 1, 64)],
                                         mdp[0:1, :], Act.Identity,
                                         bias=bmd[0:1, 0:1], scale=1.0)
                    if t >= to:
                        # d -> row 64 (ones/d row), slot t (already consumed)
                        nc.scalar.activation(xyf[64:65, ts(t, 64)],
                                             mdp[32:33, :], Act.Identity,
                                             bias=bmd[32:33, 0:1], scale=1.0)
                h1T_prev, h2T_prev = h1T, h2T

            # ---- outputs: mean row = slots TO+1..TP, disp row = slots TO..TP-1
            nc.sync.dma_start(out_d[0:1, :],
                              xyf[0:1, (to + 1) * B:(tp + 1) * B])
            nc.sync.dma_start(out_d[1:2, :],
                              xyf[64:65, to * B:tp * B])

    n = split_excess_waits(nc) if split_waits else 0
    return nc, n


_CACHE = {}


def _get_program():
    if "nc" not in _CACHE:
        _CACHE["nc"] = build_program()[0]
    return _CACHE["nc"]


def make_core_inputs(x, y, W1, b1, W2, b2, Wm, bm, Wd, bd, tp=TP, to=TO,
                     np_dt=np.float16):
    """Host-side prep: returns (in_maps list of 8 dicts, scale [512])."""
    NSLOT_ = tp + 1
    XCOLS_ = NSLOT_ * B
    x = np.asarray(x, np.float32)
    y = np.asarray(y, np.float32)
    W1 = np.asarray(W1, np.float32)
    b1 = np.asarray(b1, np.float32)
    W2 = np.asarray(W2, np.float32)
    b2 = np.asarray(b2, np.float32)
    Wm = np.asarray(Wm, np.float32)
    bm = np.asarray(bm, np.float32)
    Wd = np.asarray(Wd, np.float32)
    bd = np.asarray(bd, np.float32)

    scale = 1.0 + np.mean(y[:, 0:to, 0], axis=1)       # [512]
    y_sc = y[:, 0:to, 0] / scale[:, None]              # [512, to]

    b1a = b1.copy()
    b1a[2 * H:3 * H] += 1.0                             # forget-gate +1
    b2a = b2.copy()
    b2a[2 * H:3 * H] += 1.0

    # row layout: 0 = y/m, 1:64 = x[0:63], 64 = ones/bias (disp storage),
    # 65 = x[63]  (rows 0 and 64 must sit at legal engine partition bases)
    w1c0 = np.empty((66, G), np.float32)
    w1c0[0] = W1[F]                                     # y/m weight row
    w1c0[1:64] = W1[0:F - 1]                            # x weight rows 0..62
    w1c0[64] = b1a                                      # bias row (ones input)
    w1c0[65] = W1[F - 1]                                # x weight row 63

    w1h = np.ascontiguousarray(
        W1[F + 1:].reshape(4, 128, G).transpose(1, 0, 2).reshape(128, 4 * G))
    w2 = np.ascontiguousarray(
        W2.reshape(8, 128, G).transpose(1, 0, 2).reshape(128, 8 * G))

    wmd = np.zeros((128, 4, 64), np.float32)
    wmd[:, :, 0] = Wm[:, 0].reshape(4, 128).T
    wmd[:, :, 32] = Wd[:, 0].reshape(4, 128).T
    wmd = np.ascontiguousarray(wmd.reshape(128, 4 * 64))

    b2rep = np.ascontiguousarray(np.broadcast_to(b2a, (64, G)))
    bmd = np.zeros((33, 1), np.float32)
    bmd[0, 0] = bm[0]
    bmd[32, 0] = bd[0]
    identity = np.eye(64, dtype=np_dt)

    in_maps = []
    for c in range(NC):
        bs = slice(c * B, (c + 1) * B)
        xyf = np.zeros((66, NSLOT_, B), np.float32)
        xyf[0, 1:to, :] = y_sc[bs, 0:to - 1].T          # shifted y feed
        xt = x[bs].transpose(2, 1, 0)                   # [f, t, b]
        xyf[1:64, 0:tp, :] = xt[0:F - 1, 0:tp]          # x rows 0..62
        xyf[65, 0:tp, :] = xt[F - 1, 0:tp]              # x row 63
        xyf[64, :, :] = 1.0                             # ones / bias row
        in_maps.append({
            "xyf_d": np.ascontiguousarray(xyf.reshape(66, XCOLS_)).astype(np_dt),
            "w1c0_d": w1c0.astype(np_dt), "w1h_d": w1h.astype(np_dt),
            "w2_d": w2.astype(np_dt), "wmd_d": wmd.astype(np_dt),
            "b2r_d": b2rep, "bmd_d": bmd, "id_d": identity,
        })
    return in_maps, scale


def postprocess(results, scale, tp=TP, to=TO):
    """results: list of 8 dicts with out_d [2, (tp-to)*64] -> [512, tp-to, 2]."""
    out = np.empty((B_FULL, tp - to, 2), np.float32)
    for c in range(NC):
        r = results[c]["out_d"].astype(np.float32)
        mean_tb = r[0].reshape(tp - to, B)              # [t, b]
        dpre_tb = r[1].reshape(tp - to, B)
        bs = slice(c * B, (c + 1) * B)
        sc = scale[bs]
        out[bs, :, 0] = (mean_tb * sc[None, :]).T
        disp = np.logaddexp(dpre_tb, 0.0)               # softplus
        out[bs, :, 1] = (disp * np.sqrt(sc)[None, :]).T
    return out


def kernel(x, y, W1, b1, W2, b2, Wm, bm, Wd, bd):
    in_maps, scale = make_core_inputs(x, y, W1, b1, W2, b2, Wm, bm, Wd, bd)
    nc = _get_program()
    res = bass_utils.run_bass_kernel_spmd(nc, in_maps, core_ids=list(range(NC)))
    return postprocess(res.results, scale)
